# revision 11
# baseline (speedup 1.0000x reference)
"""GCN 2-layer kernel for Trainium2, 8 NeuronCores.

Architecture: 3 SPMD dispatches, all gathers done on host (index movement).
  - Shard by destination-node range: core c owns dst nodes [c*12544, (c+1)*12544).
  - Edges (incl. self-loops) are routed to the dst-owning core, sorted by dst,
    packed into 128-edge chunks targeting 32-node destination windows
    (global static schedule so all cores run identical code).
  - Degree is a host-side bincount of dst indices (part of schedule build);
    all FLOAT math (rsqrt etc.) stays on device.
  - d0 (tiny): dis = sqrt(1/deg), sq = sqrt(deg), ideg = 1/deg, xs = x*dis
    (bf16). Host gathers xs[src] per edge.
  - d2: one-hot cmp = (iota == dst_rel) in bf16 (DVE); scatter-add via
    col-tiled bf16 matmuls (4 concurrent 32-partition groups); psum holds
    raw aggregate A_raw[f, n] = sum_e xs_src.  The dis_d scaling is folded
    through relu via  relu(dis*x + b1) = dis*relu(x + sqrt(deg)*b1):
    phase B computes h1t = relu([W1; b1]^T @ [A_raw; sq]) (K=17 matmul),
    phase C computes zs = ideg * (h1t^T @ W2) with a per-partition scalar.
    cmp tiles are streamed out to HBM for d3 to reuse.
  - host: gathers zs[src] per edge (bf16).
  - d3: scatter-add zs_src via the HBM-cached cmp one-hots (no compares);
    out = dis_d * agg + b2 via two [128,196] tensor ops.
"""
import sys

sys.path.insert(0, '/opt/trn_rl_repo')

import numpy as np
import concourse.bass as bass
import concourse.tile as tile
from concourse import bacc, mybir
from concourse.bass_utils import run_bass_kernel_spmd

N_NODES = 100000
N_CORES = 8
NPC = 12544            # nodes per core = 98 * 128
NPAD = NPC * N_CORES   # 100352
W = 32                 # dst window width
NWIN = NPC // W        # 392 windows per core
NGRP = 4               # psum col groups (tile_position col tiling)
WPB = 64               # windows per psum bank (16 per group x 4 groups)
NBANKA = (NWIN + WPB - 1) // WPB   # 7 aggregation banks
NCOLS = NPC // 128     # 98 (wrap columns / 128-node slices)
NSB = (NCOLS + 3) // 4  # 25 superblocks of 512 nodes
F_IN = 16
F_HID = 128
F_OUT = 2
CHUNK = 128
CMP_BATCH = 32         # slots per compare op
DT = mybir.dt.float32
BF = mybir.dt.float16
NP_BF = np.float16


# ---------------------------------------------------------------- host prep

def build_schedule(edge_index):
    """Partition + sort edges, build the global static slot schedule and the
    per-node degree histogram (host-side integer index counting)."""
    src = np.asarray(edge_index[0])
    dst = np.asarray(edge_index[1])

    # degree histogram over dst (+1 self-loop per real node); pads get deg=1
    deg = np.bincount(dst, minlength=NPAD).astype(np.int64)
    deg[:N_NODES] += 1
    deg[N_NODES:] = 1

    per_core = []
    counts = np.zeros((N_CORES, NWIN), dtype=np.int64)
    for c in range(N_CORES):
        lo, hi = c * NPC, (c + 1) * NPC
        sel = (dst >= lo) & (dst < hi)
        es = src[sel].astype(np.int64)
        ed = (dst[sel] - lo).astype(np.int64)
        n_real = min(hi, N_NODES) - lo
        self_d = np.arange(n_real, dtype=np.int64)
        es = np.concatenate([es, self_d + lo])
        ed = np.concatenate([ed, self_d])
        order = np.argsort(ed, kind='stable')
        es, ed = es[order], ed[order]
        win = ed // W
        counts[c] = np.bincount(win, minlength=NWIN)
        per_core.append((es, ed))

    k_w = np.ceil(counts.max(axis=0) / CHUNK).astype(np.int64)
    k_w = np.maximum(k_w, 1)
    S_real = int(k_w.sum())
    S = ((S_real + CMP_BATCH - 1) // CMP_BATCH) * CMP_BATCH  # pad to batch mult

    # schedule: per slot -> (window, is_first_chunk_of_window, is_last)
    sched = []
    for w in range(NWIN):
        for k in range(int(k_w[w])):
            sched.append((w, k == 0, k == int(k_w[w]) - 1))

    # per-(bank, group) first/last slot, per-bank last slot
    first_bg, last_bg, last_bank = {}, {}, {}
    for s, (w, fc, lc) in enumerate(sched):
        bank, grp = w // WPB, w % NGRP
        if (bank, grp) not in first_bg:
            first_bg[(bank, grp)] = s
        last_bg[(bank, grp)] = s
        last_bank[bank] = s

    # per-core arrays [128, S]
    srcidx = np.zeros((N_CORES, S, CHUNK), dtype=np.int64)
    valid = np.zeros((N_CORES, S, CHUNK), dtype=bool)
    dst_rel = np.full((N_CORES, S, CHUNK), -1.0, dtype=np.float32)
    for c in range(N_CORES):
        es, ed = per_core[c]
        starts = np.zeros(NWIN + 1, dtype=np.int64)
        np.cumsum(counts[c], out=starts[1:])
        slot = 0
        for w in range(NWIN):
            e0, e1 = int(starts[w]), int(starts[w + 1])
            for k in range(int(k_w[w])):
                a = e0 + k * CHUNK
                b = min(e0 + (k + 1) * CHUNK, e1)
                m = max(0, b - a)
                if m > 0:
                    srcidx[c, slot, :m] = es[a:b]
                    valid[c, slot, :m] = True
                    dst_rel[c, slot, :m] = (ed[a:b] - w * W).astype(np.float32)
                slot += 1
        assert slot == S_real

    srcidx_t = np.ascontiguousarray(srcidx.transpose(0, 2, 1))      # [C,128,S]
    valid_t = np.ascontiguousarray(valid.transpose(0, 2, 1))
    dst_rel_t = np.ascontiguousarray(dst_rel.transpose(0, 2, 1))

    iota = np.tile(np.arange(W, dtype=np.float32), CMP_BATCH)       # [1024]
    iota_tiled = np.ascontiguousarray(np.broadcast_to(iota, (CHUNK, W * CMP_BATCH)))

    return dict(S=S, S_real=S_real, sched=sched, srcidx=srcidx_t, valid=valid_t,
                dst_rel=dst_rel_t, iota_tiled=iota_tiled, deg=deg,
                first_bg=first_bg, last_bg=last_bg, last_bank=last_bank)


def gather_rows(table, srcidx, valid, f):
    """host gather: msg[c, p, s*f:(s+1)*f] = table[srcidx[c,p,s]] (0 if pad)."""
    C, P, S = srcidx.shape
    out = table[srcidx.reshape(-1)].reshape(C, P, S, f)
    out[~valid] = 0
    return np.ascontiguousarray(out.reshape(C, P, S * f))


def wrap2(v):
    """[NPC] -> [128, 98] wrap layout (n = c*128 + p)."""
    return np.ascontiguousarray(v.reshape(NCOLS, 128).T)


def unwrap2(m):
    """[128, 98] -> [NPC]"""
    return np.ascontiguousarray(m.T.reshape(-1))


# ------------------------------------------------------------- bass helpers

def new_nc():
    return bacc.Bacc('TRN2', target_bir_lowering=False, debug=False,
                     num_devices=N_CORES)


# --------------------------------------------------------------- program d0

def build_d0():
    """dis = sqrt(1/deg); sq = sqrt(deg); ideg = 1/deg; xs = x * dis (bf16)."""
    nc = new_nc()
    x_in = nc.dram_tensor('x_wrap', [CHUNK, NCOLS * F_IN], DT, kind='ExternalInput')
    deg_in = nc.dram_tensor('deg_wrap', [CHUNK, NCOLS], DT, kind='ExternalInput')
    xs_out = nc.dram_tensor('xs_bf', [CHUNK, NCOLS * F_IN], BF, kind='ExternalOutput')
    dis_out = nc.dram_tensor('dis', [CHUNK, NCOLS], DT, kind='ExternalOutput')
    sq_out = nc.dram_tensor('sq_bf', [CHUNK, NCOLS], BF, kind='ExternalOutput')
    ideg_out = nc.dram_tensor('ideg', [CHUNK, NCOLS], DT, kind='ExternalOutput')

    with tile.TileContext(nc) as tc:
        with tc.tile_pool(name='p', bufs=1) as pp:
            x_t = pp.tile([CHUNK, NCOLS * F_IN], DT)
            nc.sync.dma_start(x_t[:], x_in.ap())
            deg_t = pp.tile([CHUNK, NCOLS], DT)
            nc.scalar.dma_start(deg_t[:], deg_in.ap())

            ideg_t = pp.tile([CHUNK, NCOLS], DT)
            nc.vector.reciprocal(ideg_t[:], deg_t[:])
            dis_t = pp.tile([CHUNK, NCOLS], DT)
            nc.scalar.sqrt(dis_t[:], ideg_t[:])
            sq_t = pp.tile([CHUNK, NCOLS], BF)
            nc.scalar.sqrt(sq_t[:], deg_t[:])

            xs_t = pp.tile([CHUNK, NCOLS * F_IN], BF)
            nc.vector.tensor_tensor(
                out=xs_t[:], in0=x_t[:],
                in1=dis_t[:].to_broadcast([CHUNK, NCOLS, F_IN]),
                op=mybir.AluOpType.mult)

            nc.sync.dma_start(xs_out.ap(), xs_t[:])
            nc.scalar.dma_start(dis_out.ap(), dis_t[:])
            nc.gpsimd.dma_start(sq_out.ap(), sq_t[:])
            nc.gpsimd.dma_start(ideg_out.ap(), ideg_t[:])

    nc.compile()
    return nc


# --------------------------------------------------------------- program d2

def build_d2(S, sched, first_bg, last_bg, last_bank):
    """Layer 1 + z:  A_raw scatter-add -> h1t = relu(W1b^T @ [A_raw; sq])
    -> zs = ideg * (h1t^T @ W2).  Streams cmp one-hots to HBM for d3."""
    nc = new_nc()
    dst_rel_in = nc.dram_tensor('dst_rel', [CHUNK, S], DT, kind='ExternalInput')
    iota_in = nc.dram_tensor('iota_tiled', [CHUNK, CMP_BATCH * W], DT,
                             kind='ExternalInput')
    xsrc_in = nc.dram_tensor('xs_src', [CHUNK, S * F_IN], BF, kind='ExternalInput')
    sq_in = nc.dram_tensor('sq_row', [1, NPC], BF, kind='ExternalInput')
    ideg_in = nc.dram_tensor('ideg', [CHUNK, NCOLS], DT, kind='ExternalInput')
    w1b_in = nc.dram_tensor('W1b', [F_IN + 1, F_HID], DT, kind='ExternalInput')
    w2_in = nc.dram_tensor('W2', [F_HID, F_OUT], DT, kind='ExternalInput')
    zs_out = nc.dram_tensor('zs_wrap', [CHUNK, F_OUT * NCOLS], BF,
                            kind='ExternalOutput')
    cmp_out = nc.dram_tensor('cmp_hbm', [CHUNK, S * W], BF, kind='ExternalOutput')

    n_batches = S // CMP_BATCH

    with tile.TileContext(nc) as tc:
        with tc.tile_pool(name='persist', bufs=1) as pp, \
             tc.tile_pool(name='cmp', bufs=6) as cmpp, \
             tc.tile_pool(name='msg', bufs=8) as msgp, \
             tc.tile_pool(name='aggps', bufs=3, space='PSUM') as aggps, \
             tc.tile_pool(name='h1ps', bufs=2, space='PSUM') as h1ps, \
             tc.tile_pool(name='zps', bufs=2, space='PSUM') as zps:
            dst_rel_t = pp.tile([CHUNK, S], DT)
            nc.scalar.dma_start(dst_rel_t[:], dst_rel_in.ap())
            iota_t = pp.tile([CHUNK, CMP_BATCH * W], DT)
            nc.sync.dma_start(iota_t[:], iota_in.ap())
            ideg_t = pp.tile([CHUNK, NCOLS], DT)
            nc.sync.dma_start(ideg_t[:], ideg_in.ap())
            w1b_f32 = pp.tile([F_IN + 1, F_HID], DT)
            nc.sync.dma_start(w1b_f32[:], w1b_in.ap())
            w1b_t = pp.tile([F_IN + 1, F_HID], BF)
            nc.vector.tensor_copy(w1b_t[:], w1b_f32[:])
            w2_f32 = pp.tile([F_HID, F_OUT], DT)
            nc.sync.dma_start(w2_f32[:], w2_in.ap())
            w2_t = pp.tile([F_HID, F_OUT], BF)
            nc.vector.tensor_copy(w2_t[:], w2_f32[:])

            # agg17[0:16] = raw aggregate (flushed from psum), agg17[16] = sq
            agg17 = pp.tile([F_IN + 1, NPC], BF)
            nc.sync.dma_start(agg17[F_IN:F_IN + 1, :], sq_in.ap())
            h1_sb = pp.tile([F_HID, NPC], BF)
            zs_sb = pp.tile([CHUNK, F_OUT * NCOLS], BF)

            agg_tiles = {}

            def flush_bank(bank):
                """psum bank -> agg17 rows 0..16 (ACT engine)."""
                w0 = bank * WPB
                nw = min(NWIN, w0 + WPB) - w0           # windows in bank
                nwg = nw // NGRP                        # per group
                at = agg_tiles[bank]
                for g in range(NGRP):
                    # agg17 cols for window w=4a+g, a in [16*bank, 16*bank+nwg)
                    view = (agg17[0:F_IN, 2048 * bank:2048 * bank + 128 * nwg]
                            .rearrange('p (a r) -> p a r', r=128)
                            [:, :, 32 * g:32 * g + W])
                    srcv = (at[32 * g:32 * g + F_IN, 0:32 * nwg]
                            .rearrange('p (a r) -> p a r', r=W))
                    nc.scalar.copy(view, srcv)

            def emit_B(bank):
                for k in range(4 * bank, min(4 * bank + 4, NSB)):
                    c0 = 512 * k
                    c1 = min(c0 + 512, NPC)
                    h1p = h1ps.tile([F_HID, 512], DT, space='PSUM', tag='h1')
                    nc.tensor.matmul(out=h1p[:, :c1 - c0], lhsT=w1b_t[:],
                                     rhs=agg17[:, c0:c1], start=True, stop=True)
                    nc.scalar.activation(h1_sb[:, c0:c1], h1p[:, :c1 - c0],
                                         mybir.ActivationFunctionType.Relu)

            def emit_C(bank):
                for k in range(4 * bank, min(4 * bank + 4, NSB)):
                    c0 = 512 * k
                    c1 = min(c0 + 512, NPC)
                    for sl in range(c0 // 128, c1 // 128):
                        zp = zps.tile([CHUNK, F_OUT], DT, space='PSUM', tag='z')
                        nc.tensor.matmul(out=zp[:],
                                         lhsT=h1_sb[:, sl * 128:(sl + 1) * 128],
                                         rhs=w2_t[:], start=True, stop=True)
                        nc.scalar.mul(zs_sb[:, sl * F_OUT:(sl + 1) * F_OUT],
                                      zp[:], ideg_t[:, sl:sl + 1])

            for b in range(n_batches):
                cmp_t = cmpp.tile([CHUNK, CMP_BATCH * W], BF, tag='cmp')
                nc.vector.tensor_tensor(
                    out=cmp_t[:],
                    in0=iota_t[:],
                    in1=dst_rel_t[:, b * CMP_BATCH:(b + 1) * CMP_BATCH]
                        .to_broadcast([CHUNK, CMP_BATCH, W]),
                    op=mybir.AluOpType.is_equal)
                nc.gpsimd.dma_start(
                    cmp_out.ap()[:, b * CMP_BATCH * W:(b + 1) * CMP_BATCH * W],
                    cmp_t[:])
                msg_t = msgp.tile([CHUNK, CMP_BATCH * F_IN], BF, tag='msg')
                nc.sync.dma_start(
                    msg_t[:],
                    xsrc_in.ap()[:, b * CMP_BATCH * F_IN:(b + 1) * CMP_BATCH * F_IN])
                for j in range(CMP_BATCH):
                    s = b * CMP_BATCH + j
                    if s >= len(sched):
                        break
                    w, fc, lc = sched[s]
                    bank, grp = w // WPB, w % NGRP
                    colb = 32 * ((w // NGRP) % (WPB // NGRP))
                    if bank not in agg_tiles:
                        agg_tiles[bank] = aggps.tile(
                            [CHUNK, 512], DT, space='PSUM', tag='agg',
                            name=f'aggbank{bank}')
                    nc.tensor.matmul(
                        out=agg_tiles[bank][32 * grp:32 * grp + F_IN,
                                            colb:colb + W],
                        lhsT=msg_t[:, j * F_IN:(j + 1) * F_IN],
                        rhs=cmp_t[:, j * W:(j + 1) * W],
                        start=(s == first_bg[(bank, grp)]),
                        stop=(s == last_bg[(bank, grp)]),
                        tile_position=(0, 32 * grp),
                    )
                    if s == last_bank[bank]:
                        flush_bank(bank)
                        if bank >= 1:
                            emit_B(bank - 1)
                        if bank >= 2:
                            emit_C(bank - 2)

            emit_B(NBANKA - 1)
            emit_C(NBANKA - 2)
            emit_C(NBANKA - 1)
            nc.sync.dma_start(zs_out.ap(), zs_sb[:])

    nc.compile()
    return nc


# --------------------------------------------------------------- program d3

def build_d3(S, sched):
    """Layer 2 aggregation from HBM-cached one-hots:
    out = dis_d * scatter(zs_src) + b2."""
    nc = new_nc()
    cmp_in = nc.dram_tensor('cmp_hbm', [CHUNK, S * W], BF, kind='ExternalInput')
    zssrc_in = nc.dram_tensor('zs_src', [CHUNK, S * F_OUT], BF,
                              kind='ExternalInput')
    drep_in = nc.dram_tensor('drep', [CHUNK, F_OUT * NWIN // NGRP], DT,
                             kind='ExternalInput')
    b2rep_in = nc.dram_tensor('b2rep', [CHUNK, F_OUT * NWIN // NGRP], DT,
                              kind='ExternalInput')
    out_out = nc.dram_tensor('out_wrap4', [CHUNK, F_OUT * NWIN // NGRP], DT,
                             kind='ExternalOutput')

    NCOL3 = F_OUT * NWIN // NGRP    # 196
    n_batches = S // CMP_BATCH

    with tile.TileContext(nc) as tc:
        with tc.tile_pool(name='persist', bufs=1) as pp, \
             tc.tile_pool(name='cmp', bufs=8) as cmpp, \
             tc.tile_pool(name='msg', bufs=8) as msgp, \
             tc.tile_pool(name='psum', bufs=1, space='PSUM') as psp, \
             tc.tile_pool(name='outp', bufs=1) as outp:
            drep_t = pp.tile([CHUNK, NCOL3], DT)
            nc.sync.dma_start(drep_t[:], drep_in.ap())
            b2rep_t = pp.tile([CHUNK, NCOL3], DT)
            nc.sync.dma_start(b2rep_t[:], b2rep_in.ap())

            out_ps = psp.tile([CHUNK, NCOL3], DT, space='PSUM')

            for b in range(n_batches):
                cmp_t = cmpp.tile([CHUNK, CMP_BATCH * W], BF, tag='cmp')
                eng = nc.sync if b % 2 == 0 else nc.scalar
                eng.dma_start(
                    cmp_t[:],
                    cmp_in.ap()[:, b * CMP_BATCH * W:(b + 1) * CMP_BATCH * W])
                msg_t = msgp.tile([CHUNK, CMP_BATCH * F_OUT], BF, tag='msg')
                nc.sync.dma_start(
                    msg_t[:],
                    zssrc_in.ap()[:, b * CMP_BATCH * F_OUT:(b + 1) * CMP_BATCH * F_OUT])
                for j in range(CMP_BATCH):
                    s = b * CMP_BATCH + j
                    if s >= len(sched):
                        break
                    w, fc, lc = sched[s]
                    grp = w % NGRP
                    col = F_OUT * (w // NGRP)
                    nc.tensor.matmul(
                        out=out_ps[32 * grp:32 * grp + W, col:col + F_OUT],
                        lhsT=cmp_t[:, j * W:(j + 1) * W],
                        rhs=msg_t[:, j * F_OUT:(j + 1) * F_OUT],
                        start=(s == _d3_first[grp]),
                        stop=(s == _d3_last[grp]),
                        tile_position=(0, 32 * grp),
                    )

            scaled = outp.tile([CHUNK, NCOL3], DT)
            nc.vector.tensor_tensor(out=scaled[:], in0=out_ps[:], in1=drep_t[:],
                                    op=mybir.AluOpType.mult)
            final = outp.tile([CHUNK, NCOL3], DT)
            nc.vector.tensor_tensor(out=final[:], in0=scaled[:], in1=b2rep_t[:],
                                    op=mybir.AluOpType.add)
            nc.sync.dma_start(out_out.ap(), final[:])

    nc.compile()
    return nc


_d3_first = {}
_d3_last = {}


def prep_d3_groups(sched):
    _d3_first.clear()
    _d3_last.clear()
    for s, (w, fc, lc) in enumerate(sched):
        g = w % NGRP
        if g not in _d3_first:
            _d3_first[g] = s
        _d3_last[g] = s


# ------------------------------------------------------------------ runner

RESULTS = []  # BassKernelResults of the last run (for profiling)


def run_gcn(x, edge_index, W1, b1, W2, b2, trace=False):
    x = np.asarray(x, dtype=np.float32)
    W1 = np.asarray(W1, dtype=np.float32)
    b1 = np.asarray(b1, dtype=np.float32)
    W2 = np.asarray(W2, dtype=np.float32)
    b2 = np.asarray(b2, dtype=np.float32)

    sch = build_schedule(edge_index)
    S, sched = sch['S'], sch['sched']
    prep_d3_groups(sched)
    print(f'[host] slots S={S} (real {sch["S_real"]}), '
          f'edges+selfloops={int(sch["valid"].sum())}')

    import time
    t0 = time.time()
    nc0 = build_d0()
    nc2 = build_d2(S, sched, sch['first_bg'], sch['last_bg'], sch['last_bank'])
    nc3 = build_d3(S, sched)
    print(f'[host] compiled in {time.time()-t0:.1f}s')

    core_ids = list(range(N_CORES))
    times = {}
    RESULTS.clear()

    # ---------- d0
    x_pad = np.zeros((NPAD, F_IN), dtype=np.float32)
    x_pad[:N_NODES] = x
    deg_f = sch['deg'].astype(np.float32)
    in0 = []
    for c in range(N_CORES):
        lo = c * NPC
        xw = np.ascontiguousarray(
            x_pad[lo:lo + NPC].reshape(NCOLS, 128, F_IN).transpose(1, 0, 2)
            .reshape(CHUNK, NCOLS * F_IN))
        in0.append({'x_wrap': xw, 'deg_wrap': wrap2(deg_f[lo:lo + NPC])})
    r0 = run_bass_kernel_spmd(nc0, in0, core_ids=core_ids, trace=trace)
    RESULTS.append(r0)
    times['d0'] = r0.exec_time_ns

    xs_full = np.zeros((NPAD, F_IN), dtype=NP_BF)
    dis_full = np.empty(NPAD, dtype=np.float32)
    sq_full = np.empty(NPAD, dtype=NP_BF)
    for c in range(N_CORES):
        lo = c * NPC
        xs_full[lo:lo + NPC] = (r0.results[c]['xs_bf']
                                .reshape(CHUNK, NCOLS, F_IN).transpose(1, 0, 2)
                                .reshape(NPC, F_IN))
        dis_full[lo:lo + NPC] = unwrap2(r0.results[c]['dis'])
        sq_full[lo:lo + NPC] = unwrap2(r0.results[c]['sq_bf'])
    xs_full[N_NODES:] = 0

    # ---------- host gather (index movement only)
    xs_src = gather_rows(xs_full, sch['srcidx'], sch['valid'], F_IN)

    W1b = np.concatenate([W1, b1[None, :]], axis=0)  # [17, 128]

    # ---------- d2
    in2 = []
    for c in range(N_CORES):
        lo = c * NPC
        in2.append({
            'dst_rel': sch['dst_rel'][c], 'iota_tiled': sch['iota_tiled'],
            'xs_src': xs_src[c],
            'sq_row': np.ascontiguousarray(sq_full[lo:lo + NPC])[None, :],
            'ideg': r0.results[c]['ideg'],
            'W1b': W1b, 'W2': W2,
        })
    r2 = run_bass_kernel_spmd(nc2, in2, core_ids=core_ids, trace=trace)
    RESULTS.append(r2)
    times['d2'] = r2.exec_time_ns

    zs_full = np.zeros((NPAD, F_OUT), dtype=NP_BF)
    for c in range(N_CORES):
        lo = c * NPC
        zs_full[lo:lo + NPC] = (r2.results[c]['zs_wrap']
                                .reshape(CHUNK, NCOLS, F_OUT).transpose(1, 0, 2)
                                .reshape(NPC, F_OUT))
    zs_full[N_NODES:] = 0

    zs_src = gather_rows(zs_full, sch['srcidx'], sch['valid'], F_OUT)

    # ---------- d3
    # drep[32j+r, 2a+f] = dis[32(4a+j)+r];  b2rep[p, 2a+f] = b2[f]
    NCOL3 = F_OUT * NWIN // NGRP
    jj, rr = np.divmod(np.arange(CHUNK), W)       # p = 32j+r
    aa = np.arange(NWIN // NGRP)
    loc = (32 * (4 * aa[None, :] + jj[:, None]) + rr[:, None])  # [128, 98]
    b2rep = np.ascontiguousarray(
        np.broadcast_to(b2[None, None, :], (CHUNK, NWIN // NGRP, F_OUT))
        .reshape(CHUNK, NCOL3)).astype(np.float32)
    in3 = []
    for c in range(N_CORES):
        lo = c * NPC
        drep = np.repeat(dis_full[lo:lo + NPC][loc], F_OUT, axis=1) \
            .reshape(CHUNK, NCOL3).astype(np.float32)
        in3.append({
            'cmp_hbm': r2.results[c]['cmp_hbm'],
            'zs_src': zs_src[c],
            'drep': np.ascontiguousarray(drep),
            'b2rep': b2rep,
        })
    r3 = run_bass_kernel_spmd(nc3, in3, core_ids=core_ids, trace=trace)
    RESULTS.append(r3)
    times['d3'] = r3.exec_time_ns

    out_full = np.empty((NPAD, F_OUT), dtype=np.float32)
    for c in range(N_CORES):
        ow = r3.results[c]['out_wrap4']            # [128, 196]
        # local n = 32w+r, w = 4a+j -> p = 32j+r, col = 2a+f
        n = np.arange(NPC)
        wv, rv = np.divmod(n, W)
        jv, av = wv % NGRP, wv // NGRP
        out_full[c * NPC:(c + 1) * NPC, 0] = ow[32 * jv + rv, 2 * av]
        out_full[c * NPC:(c + 1) * NPC, 1] = ow[32 * jv + rv, 2 * av + 1]
    return out_full[:N_NODES].astype(np.float32), times


# ------------------------------------------------------------- entry point

TRACE = False
LAST_TIMES = {}


def kernel(x, edge_index, W1, b1, W2, b2):
    """Full-input GCN kernel: shards across 8 NeuronCores internally."""
    global LAST_TIMES
    out, times = run_gcn(x, edge_index, W1, b1, W2, b2, trace=TRACE)
    LAST_TIMES = times
    return out
